# revision 3
# baseline (speedup 1.0000x reference)
"""DeepSeekMoE (BitNet-quantized) Trainium2 kernel — transpose-free design.

Strategy (8 NeuronCores, SPMD):
  - Host: rmsnorm + per-token int8 activation quant + router (exact bf16
    replication of the reference routing) + top-k dispatch.  Ternary weight
    quant (BitNet b1.58) shipped as fp8e4 {-1,0,+1}; activations shipped as
    q = bf16(n1/s1) with the per-token dequant scale pre-folded, so the
    device never needs a per-token scale on the free axis.
  - Core i: routed expert i on its dispatched tokens (capacity 512; the ~1%
    overflow token-units run through the exact host fallback), plus shared
    expert i//4 on token block i%4 (512 tokens).
  - Device layout is tokens-on-free throughout: fc1 produces aT = [f, tok]
    directly (stationary = w1 tile [d,128f], moving = qT [d, 512 tok]), so
    NO PE transposes and no int8 requant chain are needed.  Between layers:
    ONE Act pass, aT = bf16(Silu(psum * sc1)) with the per-expert constant
    scale.  fc2 consumes aT as the moving operand (contraction over f on
    partitions) into psum [d-chunk, tok]; a DVE copy casts to bf16 out.
  - Host: out * (gate * sc2) scatter-add in f32.

  Skipping the reference's exact int8 re-quant between fc1/fc2 costs
  rel-err 1.32e-2 (vs the 1.30e-2 floor of any non-replicating scheme and
  the 2e-2 gate) — measured on the fixed seed-0 inputs in numpy.

PE budget/core: 2 units x 256 matmuls x 213ns = 109.2us; Act ~20us,
DVE ~10us, DMA ~10MB (~29us) all hidden under the PE stream.
"""

import numpy as np
import ml_dtypes

BF16 = ml_dtypes.bfloat16
F8 = ml_dtypes.float8_e4m3
F32 = np.float32

P = 128
D_ = 1024
F_ = 2048
E_ = 8
T_ = 2048
NCORES = 8
C_ROUT = 512  # routed-token capacity; overflow runs through the host fallback
T_SH = 512    # shared-expert token block per core

KD = D_ // P   # 8  fc1 contraction tiles
KF = F_ // P   # 16 fc2 contraction tiles
NC1 = F_ // P  # 16 fc1 output chunks (f)
NC2 = D_ // P  # 8  fc2 output chunks (d)

TRACE = False
_LAST_RESULTS = None
_NC_CACHE = None


# ----------------------------------------------------------------------------
# host-side math (replicates reference.py numerics)
# ----------------------------------------------------------------------------

def _rmsnorm(x2d, w):
    ms = np.mean(x2d * x2d, axis=-1, dtype=np.float32, keepdims=True) + F32(1e-6)
    return (x2d * (F32(1.0) / np.sqrt(ms)) * w).astype(np.float32)


def _quant_a(h):
    # returns integer levels n in [-128,127] (f32) and scale s with q = n / s
    mx = np.maximum(np.abs(h).max(axis=-1), F32(1e-5)).astype(np.float32)
    s = (F32(127.0) / mx).astype(np.float32)
    n = np.clip(np.round(h * s[:, None]), -128.0, 127.0).astype(np.float32)
    return n, s


def _quant_w(w):
    # per-matrix ternary quant; returns ternary (f32 {-1,0,1}) and scale
    scale = F32(np.mean(np.abs(w), dtype=np.float32) + F32(1e-8))
    t = np.clip(np.round(w / scale), -1.0, 1.0).astype(np.float32)
    return t, scale


def _route(h, router_w, top_k):
    hb = h.astype(BF16).astype(np.float32)
    rb = router_w.astype(BF16).astype(np.float32)
    logits = (hb @ rb.T).astype(BF16).astype(np.float32)
    m = logits.max(-1, keepdims=True)
    p = np.exp(logits - m)
    p /= p.sum(-1, keepdims=True)
    order = np.argsort(-p, axis=-1, kind="stable")
    idx = order[:, :top_k]
    g = np.take_along_axis(p, idx, -1)
    g = (g / g.sum(-1, keepdims=True)).astype(np.float32)
    return idx, g


def _silu(x):
    return x / (1.0 + np.exp(-x))


def _expert_mlp_rows(nq, s1, t1, sc1, t2, sc2):
    # exact numpy replication of one expert on quantized rows (fallback path)
    a = (nq / s1[:, None]) @ (t1 * sc1)
    a = _silu(a).astype(np.float32)
    mx = np.maximum(np.abs(a).max(axis=-1), F32(1e-5)).astype(np.float32)
    s2 = (F32(127.0) / mx).astype(np.float32)
    n2 = np.clip(np.round(a * s2[:, None]), -128.0, 127.0).astype(np.float32)
    return ((n2 / s2[:, None]) @ (t2 * sc2)).astype(np.float32)


# ----------------------------------------------------------------------------
# device kernel
# ----------------------------------------------------------------------------

def _build_nc(loop_n=None):
    from concourse import bacc, mybir, tile

    dt = mybir.dt
    AF = mybir.ActivationFunctionType

    nc = bacc.Bacc("TRN2", target_bir_lowering=False, debug=False,
                   num_devices=NCORES)

    def din(name, shape, dtype):
        return nc.dram_tensor(name, shape, dtype, kind="ExternalInput").ap()

    # host-tiled layouts, flattened to 2D; every per-chunk DMA is contiguous
    # on both sides (128 descriptors of >=512B).
    q_r = din("q_r", [P, KD * C_ROUT], dt.bfloat16)      # [d_in, k, t]
    q_s = din("q_s", [P, KD * T_SH], dt.bfloat16)
    w1r = din("w1r", [P, NC1 * KD * P], dt.float8e4)     # [d_in, c, k, f_in]
    w1s = din("w1s", [P, NC1 * KD * P], dt.float8e4)
    w2r = din("w2r", [P, NC2 * KF * P], dt.float8e4)     # [f_in, dc, k, d_in]
    w2s = din("w2s", [P, NC2 * KF * P], dt.float8e4)
    sc1r = din("sc1r", [P, 1], dt.float32)               # fc1 dequant scale
    sc1s = din("sc1s", [P, 1], dt.float32)

    out_r = nc.dram_tensor("out_r", [P, NC2 * C_ROUT], dt.bfloat16,
                           kind="ExternalOutput").ap()   # [d_in, dc, t]
    out_s = nc.dram_tensor("out_s", [P, NC2 * T_SH], dt.bfloat16,
                           kind="ExternalOutput").ap()

    import contextlib

    with tile.TileContext(nc) as tc:
        with (
            tc.tile_pool(name="qpool", bufs=1) as qpool,
            tc.tile_pool(name="wpool", bufs=1) as wpool,
            tc.tile_pool(name="apool", bufs=1) as apool,
            tc.tile_pool(name="spool", bufs=1) as spool,
            tc.tile_pool(name="opool", bufs=3) as opool,
            tc.tile_pool(name="pp1", bufs=2, space="PSUM") as pp1,
            tc.tile_pool(name="pp2", bufs=2, space="PSUM") as pp2,
            (tc.For_i(0, loop_n, 1,
                      hint_engines=(mybir.EngineType.PE,
                                    mybir.EngineType.DVE,
                                    mybir.EngineType.Activation,
                                    mybir.EngineType.SP))
             if loop_n is not None else contextlib.nullcontext()),
        ):
            # SBUF tensors
            q_sb = [qpool.tile([P, KD, C_ROUT], dt.bfloat16, tag="q",
                               bufs=2, name=f"q{u}") for u in range(2)]
            w1_sb = [wpool.tile([P, NC1, KD, P], dt.float8e4, tag="w1",
                                bufs=2, name=f"w1_{u}") for u in range(2)]
            w2_sb = [wpool.tile([P, NC2, KF, P], dt.float8e4, tag="w2",
                                bufs=2, name=f"w2_{u}") for u in range(2)]
            a_sb = [apool.tile([P, KF, C_ROUT], dt.bfloat16, tag="aT",
                               bufs=2, name=f"aT{u}") for u in range(2)]
            sc_sb = [spool.tile([P, 1], dt.float32, tag="sc",
                                bufs=2, name=f"sc{u}") for u in range(2)]

            q4 = [q_r.rearrange("p (k t) -> p k t", k=KD),
                  q_s.rearrange("p (k t) -> p k t", k=KD)]
            w14 = [w1r.rearrange("p (c k f) -> p c k f", c=NC1, k=KD),
                   w1s.rearrange("p (c k f) -> p c k f", c=NC1, k=KD)]
            w24 = [w2r.rearrange("p (d k f) -> p d k f", d=NC2, k=KF),
                   w2s.rearrange("p (d k f) -> p d k f", d=NC2, k=KF)]
            sc_d = [sc1r, sc1s]
            out_d = [out_r.rearrange("p (d t) -> p d t", d=NC2),
                     out_s.rearrange("p (d t) -> p d t", d=NC2)]

            # DMA feed, consumption-ordered on the SP queue.
            nc.sync.dma_start(w1_sb[0][:, 0], w14[0][:, 0])
            nc.sync.dma_start(sc_sb[0][:], sc_d[0][:])
            nc.sync.dma_start(sc_sb[1][:], sc_d[1][:])
            for k in range(KD):
                nc.sync.dma_start(q_sb[0][:, k], q4[0][:, k])
            for c in range(1, NC1):
                nc.sync.dma_start(w1_sb[0][:, c], w14[0][:, c])
            for k in range(KD):
                nc.sync.dma_start(q_sb[1][:, k], q4[1][:, k])
            for c in range(NC1):
                nc.sync.dma_start(w1_sb[1][:, c], w14[1][:, c])
            for dc in range(NC2):
                nc.sync.dma_start(w2_sb[0][:, dc], w24[0][:, dc])
            for dc in range(NC2):
                nc.sync.dma_start(w2_sb[1][:, dc], w24[1][:, dc])

            def fc1_group(u, c):
                ps1 = pp1.tile([P, C_ROUT], dt.float32, tag="ps1", name="ps1")
                for k in range(KD):
                    nc.tensor.matmul(ps1[:], w1_sb[u][:, c, k, :],
                                     q_sb[u][:, k, :],
                                     start=(k == 0), stop=(k == KD - 1))
                # aT chunk: one Act pass, constant per-expert dequant scale
                nc.scalar.activation(a_sb[u][:, c, :], ps1[:], AF.Silu,
                                     scale=sc_sb[u][:])

            def fc2_group(u, dc):
                ps2 = pp2.tile([P, C_ROUT], dt.float32, tag="ps2", name="ps2")
                for k in range(KF):
                    nc.tensor.matmul(ps2[:], w2_sb[u][:, dc, k, :],
                                     a_sb[u][:, k, :],
                                     start=(k == 0), stop=(k == KF - 1))
                osb = opool.tile([P, C_ROUT], dt.bfloat16, tag="osb",
                                 name="osb")
                nc.vector.tensor_copy(osb[:], ps2[:])
                nc.sync.dma_start(out_d[u][:, dc], osb[:])

            # PE emission order: fc1(u0) | interleave fc1(u1)+fc2(u0) | fc2(u1)
            for c in range(NC1):
                fc1_group(0, c)
            for i in range(NC2):
                fc1_group(1, 2 * i)
                fc1_group(1, 2 * i + 1)
                fc2_group(0, i)
            for dc in range(NC2):
                fc2_group(1, dc)

    nc.compile()
    return nc


def _get_nc():
    global _NC_CACHE
    if _NC_CACHE is None:
        _NC_CACHE = _build_nc()
    return _NC_CACHE


# ----------------------------------------------------------------------------
# entry point
# ----------------------------------------------------------------------------

def _prepare(x, rms_w, w1_shared, w2_shared, w1_routed, w2_routed, router_w,
             top_k):
    x = np.asarray(x)
    B, S, D = x.shape
    T = B * S
    E = np.asarray(router_w).shape[0]
    SH = np.asarray(w1_shared).shape[0]
    k_ = int(top_k)
    assert (T, D, E, SH) == (T_, D_, E_, 2) and k_ == 2

    h = _rmsnorm(x.reshape(T, D).astype(np.float32), np.asarray(rms_w))
    n1, s1 = _quant_a(h)
    idx, g = _route(h, np.asarray(router_w), k_)

    # dequantized activations, bf16 (per-token scale folded on host)
    q_bf = (n1 / s1[:, None]).astype(BF16)

    # ternary weights + scales
    t1r, sc1r, t2r, sc2r = [], [], [], []
    for e in range(E):
        t, s = _quant_w(np.asarray(w1_routed)[e]); t1r.append(t); sc1r.append(s)
        t, s = _quant_w(np.asarray(w2_routed)[e]); t2r.append(t); sc2r.append(s)
    t1s, sc1s_, t2s, sc2s_ = [], [], [], []
    for e in range(SH):
        t, s = _quant_w(np.asarray(w1_shared)[e]); t1s.append(t); sc1s_.append(s)
        t, s = _quant_w(np.asarray(w2_shared)[e]); t2s.append(t); sc2s_.append(s)

    # dispatch: token lists per expert (ascending order)
    tok_lists = [np.where((idx == e).any(axis=1))[0] for e in range(E)]
    gate_of = np.zeros((T, E), dtype=np.float32)
    for slot in range(k_):
        gate_of[np.arange(T), idx[:, slot]] += g[:, slot]

    def tile_q(qrows):  # [Tk, D] -> [128, KD*Tk]: (d_in, k, t)
        tk = qrows.shape[0]
        return np.ascontiguousarray(
            qrows.T.reshape(KD, P, tk).transpose(1, 0, 2)).reshape(P, -1)

    def tile_w1(t1):  # [D, F] -> [128, NC1*KD*128]: (d_in, c, k, f_in)
        return np.ascontiguousarray(
            t1.reshape(KD, P, NC1, P).transpose(1, 2, 0, 3)).reshape(P, -1)

    def tile_w2(t2):  # [F, D] -> [128, NC2*KF*128]: (f_in, dc, k, d_in)
        return np.ascontiguousarray(
            t2.reshape(KF, P, NC2, P).transpose(1, 2, 0, 3)).reshape(P, -1)

    in_maps = []
    for i in range(NCORES):
        toks = tok_lists[i][:C_ROUT]
        nct = len(toks)
        qr = np.zeros((C_ROUT, D_), dtype=BF16)
        qr[:nct] = q_bf[toks]

        sh, blk = i // 4, i % 4
        btok = slice(blk * T_SH, (blk + 1) * T_SH)

        in_maps.append({
            "q_r": tile_q(qr),
            "q_s": tile_q(np.ascontiguousarray(q_bf[btok])),
            "w1r": tile_w1(t1r[i].astype(F8)),
            "w1s": tile_w1(t1s[sh].astype(F8)),
            "w2r": tile_w2(t2r[i].astype(F8)),
            "w2s": tile_w2(t2s[sh].astype(F8)),
            "sc1r": np.full((P, 1), sc1r[i], dtype=np.float32),
            "sc1s": np.full((P, 1), sc1s_[sh], dtype=np.float32),
        })

    meta = {
        "B": B, "S": S, "T": T,
        "tok_lists": tok_lists, "gate_of": gate_of,
        "n1": n1, "s1": s1, "t1r": t1r, "sc1r": sc1r,
        "t2r": t2r, "sc2r": sc2r, "sc2s": sc2s_,
    }
    return in_maps, meta


def _untile_out(om, tk):  # [128, NC2*tk] -> [tk, D]
    o = np.asarray(om, dtype=np.float32).reshape(P, NC2, tk)
    return o.transpose(2, 1, 0).reshape(tk, D_)


def _assemble(results, meta):
    T = meta["T"]
    tok_lists = meta["tok_lists"]
    acc = np.zeros((T, D_), dtype=np.float32)
    for i in range(NCORES):
        om = results[i]
        sh, blk = i // 4, i % 4
        acc[blk * T_SH:(blk + 1) * T_SH] += \
            _untile_out(om["out_s"], T_SH) * meta["sc2s"][sh]
        toks = tok_lists[i][:C_ROUT]
        o_r = _untile_out(om["out_r"], C_ROUT)[:len(toks)]
        np.add.at(acc, toks,
                  o_r * (meta["gate_of"][toks, i] * meta["sc2r"][i])[:, None])
        # capacity-overflow fallback (exact reference replication)
        if len(tok_lists[i]) > C_ROUT:
            extra = tok_lists[i][C_ROUT:]
            out_e = _expert_mlp_rows(
                meta["n1"][extra], meta["s1"][extra], meta["t1r"][i],
                meta["sc1r"][i], meta["t2r"][i], meta["sc2r"][i])
            acc[extra] += meta["gate_of"][extra, i][:, None] * out_e
    return acc.reshape(meta["B"], meta["S"], D_).astype(np.float32)


def kernel(x, rms_w, w1_shared, w2_shared, w1_routed, w2_routed, router_w,
           top_k):
    global _LAST_RESULTS
    in_maps, meta = _prepare(x, rms_w, w1_shared, w2_shared, w1_routed,
                             w2_routed, router_w, top_k)
    from concourse import bass_utils
    nc = _get_nc()
    res = bass_utils.run_bass_kernel_spmd(
        nc, in_maps, core_ids=list(range(NCORES)), trace=TRACE)
    _LAST_RESULTS = res
    return _assemble(res.results, meta)
